# revision 21
# baseline (speedup 1.0000x reference)
"""MoE routing kernel for Trainium2, SPMD over 8 NeuronCores.

Math (faithful to the reference):
    ws[e, i]   = sum_o experts_weights[e, i, o]                      # [E, I]
    logits     = x @ gate_weight.T + gate_bias                       # [B, E]
    p          = softmax(logits, axis=-1)
    G[b, e]    = p[b, e] if e in top2(p[b, :]) else 0                # [B, E]
    out[b, :]  = x[b, :] * (G @ ws)[b, :] + (G @ experts_bias)[b, :]

Sharding:
  Launch 1 (reduce): experts sharded 4/core -> partial ws, gathered on host.
  Launch 2 (moe):    tokens sharded 1024/core, ws/gate weights replicated.
"""

import numpy as np

import concourse.bass as bass
import concourse.bacc as bacc
import concourse.tile as tile
from concourse import mybir
from concourse.bass_utils import run_bass_kernel_spmd
from concourse.masks import make_identity

FP32 = mybir.dt.float32
N_CORES = 8
E = 32          # num experts
I = 1024        # input dim
O = 1024        # output dim
B = 8192        # batch (tokens)
EPC = E // N_CORES   # experts per core
TPC = B // N_CORES   # tokens per core

_NC_CACHE = {}
LAST_RESULTS = []    # BassKernelResults of the most recent kernel() call
TRACE = False        # set True (e.g. by test.py) to profile the launches


# ------------------------------------------------------------------ launch 1
def build_reduce_nc():
    """Per core: w [EPC*I, O] -> wsum [32, 128] (= flat [EPC*I] row sums).

    wsum[t, p] = sum_o w[128*t + p, o]
    """
    nc = bacc.Bacc(
        "TRN2", target_bir_lowering=False, debug=False, num_devices=N_CORES
    )
    w = nc.dram_tensor("w", [EPC * I, O], FP32, kind="ExternalInput")
    wsum = nc.dram_tensor("wsum", [32, 128], FP32, kind="ExternalOutput")

    SUB = 4  # row-tiles per DMA -> [128, SUB*1024] = 2 MB transfers
    n_dma = (EPC * I) // (SUB * 128)  # 8

    with tile.TileContext(nc) as tc:
        with (
            tc.tile_pool(name="wt", bufs=4) as wpool,
            tc.tile_pool(name="acc", bufs=1) as apool,
            tc.tile_pool(name="ps", bufs=1, space="PSUM") as pspool,
        ):
            ws_sb = apool.tile([128, 32], FP32)
            junk = apool.tile([128, O], FP32)
            ident = apool.tile([128, 128], FP32)
            make_identity(nc, ident[:, :])

            for u in range(n_dma):
                wt = wpool.tile([128, SUB, O], FP32)
                src = w[u * SUB * 128 : (u + 1) * SUB * 128, :].rearrange(
                    "(s p) o -> p s o", p=128
                )
                nc.sync.dma_start(out=wt[:, :, :], in_=src)
                for s in range(SUB):
                    t = u * SUB + s
                    if s % 2 == 0:
                        nc.vector.reduce_sum(
                            ws_sb[:, t : t + 1],
                            wt[:, s, :],
                            axis=mybir.AxisListType.X,
                        )
                    else:
                        # ACT row-sum: Copy with accumulate side-output
                        nc.scalar.activation(
                            junk[:, :],
                            wt[:, s, :],
                            mybir.ActivationFunctionType.Copy,
                            bias=0.0,
                            scale=1.0,
                            accum_out=ws_sb[:, t : t + 1],
                        )

            wsT = pspool.tile([32, 128], FP32)
            nc.tensor.transpose(wsT[:, :], ws_sb[:, :], ident[:, :])
            wsT_sb = apool.tile([32, 128], FP32)
            nc.any.tensor_copy(out=wsT_sb[:, :], in_=wsT[:, :])
            nc.sync.dma_start(out=wsum[:, :], in_=wsT_sb[:, :])

    nc.compile()
    return nc


# ------------------------------------------------------------------ launch 2
def build_moe_nc(k=2, with_bias=False):
    """Per core: x [TPC, I] -> y [TPC, O], gating + top-k combine."""
    nc = bacc.Bacc(
        "TRN2", target_bir_lowering=False, debug=False, num_devices=N_CORES
    )
    x = nc.dram_tensor("x", [TPC, I], FP32, kind="ExternalInput")
    gwt = nc.dram_tensor("gwt", [128, I // 128, E], FP32, kind="ExternalInput")
    gb = nc.dram_tensor("gb", [1, E], FP32, kind="ExternalInput")
    ws = nc.dram_tensor("ws", [E, I], FP32, kind="ExternalInput")
    if with_bias:
        eb = nc.dram_tensor("eb", [E, O], FP32, kind="ExternalInput")
    y = nc.dram_tensor("y", [TPC, O], FP32, kind="ExternalOutput")

    NCH = I // 128          # 8 contraction chunks for gating
    BLK = 4                 # token-tiles per block (512 tokens, 2 MB DMA)
    n_blk = TPC // (BLK * 128)  # 2

    with tile.TileContext(nc) as tc:
        with (
            tc.tile_pool(name="const", bufs=1) as cpool,
            tc.tile_pool(name="xin", bufs=2) as xpool,
            tc.tile_pool(name="xt", bufs=2) as xtpool,
            tc.tile_pool(name="lgs", bufs=2) as lgspool,
            tc.tile_pool(name="oout", bufs=2) as opool,
            tc.tile_pool(name="gt", bufs=2) as gtpool,
            tc.tile_pool(name="sm", bufs=6) as smpool,
            tc.tile_pool(name="ptr", bufs=2, space="PSUM") as ptrpool,
            tc.tile_pool(name="plg", bufs=1, space="PSUM") as plgpool,
            tc.tile_pool(name="psm", bufs=2, space="PSUM") as psmpool,
            tc.tile_pool(name="pgt2", bufs=1, space="PSUM") as pgtpool,
            tc.tile_pool(name="pc", bufs=2, space="PSUM") as pcpool,
        ):
            ident = cpool.tile([128, 128], FP32)
            make_identity(nc, ident[:, :])
            ones = cpool.tile([1, 512], FP32)
            nc.vector.memset(ones[:, :], 1.0)
            gwt_sb = cpool.tile([128, NCH, E], FP32)
            nc.sync.dma_start(out=gwt_sb[:, :, :], in_=gwt[:, :, :])
            gb_sb = cpool.tile([1, E], FP32)
            nc.sync.dma_start(out=gb_sb[:, :], in_=gb[:, :])
            ws_sb = cpool.tile([E, I], FP32)
            nc.sync.dma_start(out=ws_sb[:, :], in_=ws[:, :])
            if with_bias:
                eb_sb = cpool.tile([E, O], FP32)
                nc.sync.dma_start(out=eb_sb[:, :], in_=eb[:, :])

            for u in range(n_blk):
                # block of 512 tokens: x natural [128, 4, 1024]
                xt = xpool.tile([128, BLK, I], FP32)
                nc.sync.dma_start(
                    out=xt[:, :, :],
                    in_=x[u * BLK * 128 : (u + 1) * BLK * 128, :].rearrange(
                        "(s p) i -> p s i", p=128
                    ),
                )
                out_sb = opool.tile([128, BLK, O], FP32)

                # ---- transpose x: xT[c][:, s*128+t] = x[t, 128c+p] for the
                # whole 512-token block, chunked by i (PE, 4 per PSUM bank)
                xT_sb = xtpool.tile([128, NCH, BLK * 128], FP32)
                for s in range(BLK):
                    for h in range(NCH // 4):
                        ptr_t = ptrpool.tile([128, 4, 128], FP32)
                        for q in range(4):
                            c = 4 * h + q
                            nc.tensor.transpose(
                                ptr_t[:, q, :],
                                xt[:, s, c * 128 : (c + 1) * 128],
                                ident[:, :],
                            )
                        nc.any.tensor_copy(
                            out=xT_sb[:, 4 * h : 4 * h + 4, s * 128 : (s + 1) * 128],
                            in_=ptr_t[:, :, :],
                        )

                # ---- gating: logitsT [E, 512] = gwT.T @ xT + gb x ones
                lgT = plgpool.tile([E, BLK * 128], FP32)
                for c in range(NCH):
                    nc.tensor.matmul(
                        lgT[:, :],
                        lhsT=gwt_sb[:, c, :],
                        rhs=xT_sb[:, c, :],
                        start=(c == 0),
                        stop=False,
                    )
                nc.tensor.matmul(
                    lgT[:, :],
                    lhsT=gb_sb[:1, :],
                    rhs=ones[:1, :],
                    start=False,
                    stop=True,
                )
                lgT_sb = lgspool.tile([E, BLK * 128], FP32)
                nc.any.tensor_copy(out=lgT_sb[:, :], in_=lgT[:, :])

                for s in range(BLK):
                    # ---- logits [128 t, E] = transpose back (PE)
                    lg = psmpool.tile([128, E], FP32, tag="lg")
                    nc.tensor.transpose(
                        lg[:, :],
                        lgT_sb[:, s * 128 : (s + 1) * 128],
                        ident[:E, :E],
                    )
                    # ---- softmax + top-k mask; |logits| <= ~10 so skip
                    # the max-subtraction (exp is exact-safe in fp32)
                    ex = smpool.tile([128, E], FP32, tag="ex")
                    ssum = smpool.tile([128, 1], FP32, tag="ssum")
                    nc.scalar.activation(
                        ex[:, :],
                        lg[:, :],
                        mybir.ActivationFunctionType.Exp,
                        bias=0.0,
                        scale=1.0,
                        accum_out=ssum[:, :],
                    )
                    rcp = smpool.tile([128, 1], FP32, tag="rcp")
                    nc.vector.reciprocal(rcp[:, :], ssum[:, :])
                    p = smpool.tile([128, E], FP32, tag="p")
                    nc.vector.tensor_scalar_mul(p[:, :], ex[:, :], rcp[:, :])
                    mx = smpool.tile([128, 8], FP32, tag="mx")
                    nc.vector.max(out=mx[:, :], in_=p[:, :])
                    if k < 8:
                        nc.vector.memset(mx[:, k:], 0.0)
                    pz = smpool.tile([128, E], FP32, tag="pz")
                    nc.vector.match_replace(
                        out=pz[:, :],
                        in_to_replace=mx[:, :],
                        in_values=p[:, :],
                        imm_value=0.0,
                    )
                    g = smpool.tile([128, E], FP32, tag="g")
                    nc.vector.tensor_sub(g[:, :], p[:, :], pz[:, :])
                    # ---- transpose G -> [E, 128]
                    pgt = pgtpool.tile([E, 128], FP32)
                    nc.tensor.transpose(pgt[:, :], g[:, :], ident[:, :])
                    gT = gtpool.tile([E, 128], FP32)
                    nc.any.tensor_copy(out=gT[:, :], in_=pgt[:, :])
                    # ---- combine: out = x * (G @ ws) [+ G @ eb]
                    for n in range(O // 512):
                        pc = pcpool.tile([128, 512], FP32, tag="pc")
                        nc.tensor.matmul(
                            pc[:, :],
                            lhsT=gT[:, :],
                            rhs=ws_sb[:, n * 512 : (n + 1) * 512],
                            start=True,
                            stop=True,
                        )
                        nc.vector.tensor_mul(
                            out_sb[:, s, n * 512 : (n + 1) * 512],
                            xt[:, s, n * 512 : (n + 1) * 512],
                            pc[:, :],
                        )
                        if with_bias:
                            pc2 = pcpool.tile([128, 512], FP32, tag="pc2")
                            nc.tensor.matmul(
                                pc2[:, :],
                                lhsT=gT[:, :],
                                rhs=eb_sb[:, n * 512 : (n + 1) * 512],
                                start=True,
                                stop=True,
                            )
                            nc.vector.tensor_add(
                                out_sb[:, s, n * 512 : (n + 1) * 512],
                                out_sb[:, s, n * 512 : (n + 1) * 512],
                                pc2[:, :],
                            )

                nc.sync.dma_start(
                    out=y[u * BLK * 128 : (u + 1) * BLK * 128, :].rearrange(
                        "(s p) i -> p s i", p=128
                    ),
                    in_=out_sb[:, :, :],
                )

    nc.compile()
    return nc


# ------------------------------------------------------------- merged launch
def build_merged_nc(k=2, with_bias=False):
    """Single launch per core: W-shard reduce + AllGather(ws) + MoE.

    Core d inputs: w = experts_weights[4d:4d+4] flat [4096, 1024],
    x = tokens [1024d:1024(d+1)] [1024, 1024], gwt/gb replicated.
    The 16 MB W stream (sync ring) hides the PE-heavy gating work;
    combine matmuls wait on the tiny AllGather of ws.
    """
    nc = bacc.Bacc(
        "TRN2", target_bir_lowering=False, debug=False, num_devices=N_CORES
    )
    w = nc.dram_tensor("w", [EPC * I, O], FP32, kind="ExternalInput")
    x = nc.dram_tensor("x", [TPC, I], FP32, kind="ExternalInput")
    gwt = nc.dram_tensor("gwt", [128, I // 128, E], FP32, kind="ExternalInput")
    gb = nc.dram_tensor("gb", [1, E], FP32, kind="ExternalInput")
    if with_bias:
        eb = nc.dram_tensor("eb", [E, O], FP32, kind="ExternalInput")
    y = nc.dram_tensor("y", [TPC, O], FP32, kind="ExternalOutput")

    SUB = 4                      # W row-tiles per DMA (2 MB)
    n_dma = (EPC * I) // (SUB * 128)   # 8
    NCH = I // 128               # 8
    BLK = 4                      # token-tiles per block
    n_blk = TPC // (BLK * 128)   # 2

    with tile.TileContext(nc) as tc:
        with (
            tc.tile_pool(name="wt", bufs=4) as wpool,
            tc.tile_pool(name="const", bufs=1) as cpool,
            tc.tile_pool(name="dram", bufs=1, space="DRAM") as dpool,
            tc.tile_pool(name="xin", bufs=2) as xpool,
            tc.tile_pool(name="xt", bufs=2) as xtpool,
            tc.tile_pool(name="lgs", bufs=2) as lgspool,
            tc.tile_pool(name="oout", bufs=2) as opool,
            tc.tile_pool(name="gt", bufs=2) as gtpool,
            tc.tile_pool(name="sm", bufs=6) as smpool,
            tc.tile_pool(name="ptr", bufs=2, space="PSUM") as ptrpool,
            tc.tile_pool(name="plg", bufs=2, space="PSUM") as plgpool,
            tc.tile_pool(name="psm", bufs=1, space="PSUM") as psmpool,
            tc.tile_pool(name="pc", bufs=2, space="PSUM") as pcpool,
        ):
            # ---------- constants (scalar ring; sync ring is for W)
            ident = cpool.tile([128, 128], FP32)
            make_identity(nc, ident[:, :])
            ones = cpool.tile([1, 512], FP32)
            nc.vector.memset(ones[:, :], 1.0)
            gwt_sb = cpool.tile([128, NCH, E], FP32)
            nc.scalar.dma_start(out=gwt_sb[:, :, :], in_=gwt[:, :, :])
            gb_sb = cpool.tile([1, E], FP32)
            nc.scalar.dma_start(out=gb_sb[:, :], in_=gb[:, :])
            if with_bias:
                eb_sb = cpool.tile([E, O], FP32)
                nc.scalar.dma_start(out=eb_sb[:, :], in_=eb[:, :])

            # ---------- phase A: ws_part = row sums of this core's W shard
            ws_acc = cpool.tile([128, 32], FP32)
            junk = cpool.tile([128, O], FP32)
            for u in range(n_dma):
                wt = wpool.tile([128, SUB, O], FP32)
                nc.sync.dma_start(
                    out=wt[:, :, :],
                    in_=w[u * SUB * 128 : (u + 1) * SUB * 128, :].rearrange(
                        "(s p) o -> p s o", p=128
                    ),
                )
                for s in range(SUB):
                    t = u * SUB + s
                    if s % 2 == 0:
                        nc.vector.reduce_sum(
                            ws_acc[:, t : t + 1],
                            wt[:, s, :],
                            axis=mybir.AxisListType.X,
                        )
                    else:
                        nc.scalar.activation(
                            junk[:, :],
                            wt[:, s, :],
                            mybir.ActivationFunctionType.Copy,
                            bias=0.0,
                            scale=1.0,
                            accum_out=ws_acc[:, t : t + 1],
                        )
            wsT = psmpool.tile([32, 128], FP32, tag="pgt")
            nc.tensor.transpose(wsT[:, :], ws_acc[:, :], ident[:, :])
            wsT_sb = cpool.tile([32, 128], FP32)
            nc.any.tensor_copy(out=wsT_sb[:, :], in_=wsT[:, :])

            # ---------- AllGather ws across the 8 cores
            ws_bounce = dpool.tile([32, 128], FP32)
            ws_all = dpool.tile([N_CORES * 32, 128], FP32)
            nc.sync.dma_start(out=ws_bounce[:, :], in_=wsT_sb[:, :])
            nc.gpsimd.collective_compute(
                "AllGather",
                mybir.AluOpType.bypass,
                ins=[ws_bounce[:, :]],
                outs=[ws_all[:, :]],
                replica_groups=[list(range(N_CORES))],
            )
            ws_sb = cpool.tile([E, I], FP32)
            nc.sync.dma_start(
                out=ws_sb[:, :].rearrange("e (a p) -> e a p", p=128),
                in_=ws_all[:, :].rearrange("(e a) p -> e a p", a=I // 128),
            )

            # ---------- phase B: gating + top-k combine (token shard)
            for u in range(n_blk):
                xt = xpool.tile([128, BLK, I], FP32)
                nc.scalar.dma_start(
                    out=xt[:, :, :],
                    in_=x[u * BLK * 128 : (u + 1) * BLK * 128, :].rearrange(
                        "(s p) i -> p s i", p=128
                    ),
                )
                out_sb = opool.tile([128, BLK, O], FP32)

                xT_sb = xtpool.tile([128, NCH, BLK * 128], FP32)
                for s in range(BLK):
                    for h in range(NCH // 4):
                        ptr_t = ptrpool.tile([128, 4, 128], FP32)
                        for q in range(4):
                            c = 4 * h + q
                            nc.tensor.transpose(
                                ptr_t[:, q, :],
                                xt[:, s, c * 128 : (c + 1) * 128],
                                ident[:, :],
                            )
                        nc.any.tensor_copy(
                            out=xT_sb[
                                :, 4 * h : 4 * h + 4, s * 128 : (s + 1) * 128
                            ],
                            in_=ptr_t[:, :, :],
                        )

                lgT = plgpool.tile([E, BLK * 128], FP32)
                for c in range(NCH):
                    nc.tensor.matmul(
                        lgT[:, :],
                        lhsT=gwt_sb[:, c, :],
                        rhs=xT_sb[:, c, :],
                        start=(c == 0),
                        stop=False,
                    )
                nc.tensor.matmul(
                    lgT[:, :],
                    lhsT=gb_sb[:1, :],
                    rhs=ones[:1, :],
                    start=False,
                    stop=True,
                )
                lgT_sb = lgspool.tile([E, BLK * 128], FP32)
                nc.any.tensor_copy(out=lgT_sb[:, :], in_=lgT[:, :])

                for s in range(BLK):
                    lg = psmpool.tile([128, E], FP32, tag="lg")
                    nc.tensor.transpose(
                        lg[:, :],
                        lgT_sb[:, s * 128 : (s + 1) * 128],
                        ident[:E, :E],
                    )
                    ex = smpool.tile([128, E], FP32, tag="ex")
                    ssum = smpool.tile([128, 1], FP32, tag="ssum")
                    nc.scalar.activation(
                        ex[:, :],
                        lg[:, :],
                        mybir.ActivationFunctionType.Exp,
                        bias=0.0,
                        scale=1.0,
                        accum_out=ssum[:, :],
                    )
                    rcp = smpool.tile([128, 1], FP32, tag="rcp")
                    nc.vector.reciprocal(rcp[:, :], ssum[:, :])
                    p = smpool.tile([128, E], FP32, tag="p")
                    nc.vector.tensor_scalar_mul(p[:, :], ex[:, :], rcp[:, :])
                    mx = smpool.tile([128, 8], FP32, tag="mx")
                    nc.vector.max(out=mx[:, :], in_=p[:, :])
                    if k < 8:
                        nc.vector.memset(mx[:, k:], 0.0)
                    pz = smpool.tile([128, E], FP32, tag="pz")
                    nc.vector.match_replace(
                        out=pz[:, :],
                        in_to_replace=mx[:, :],
                        in_values=p[:, :],
                        imm_value=0.0,
                    )
                    g = smpool.tile([128, E], FP32, tag="g")
                    nc.vector.tensor_sub(g[:, :], p[:, :], pz[:, :])
                    pgt = psmpool.tile([E, 128], FP32, tag="pgt")
                    nc.tensor.transpose(pgt[:, :], g[:, :], ident[:, :])
                    gT = gtpool.tile([E, 128], FP32)
                    nc.any.tensor_copy(out=gT[:, :], in_=pgt[:, :])
                    for n in range(O // 512):
                        pc = pcpool.tile([128, 512], FP32, tag="pc")
                        nc.tensor.matmul(
                            pc[:, :],
                            lhsT=gT[:, :],
                            rhs=ws_sb[:, n * 512 : (n + 1) * 512],
                            start=True,
                            stop=True,
                        )
                        nc.vector.tensor_mul(
                            out_sb[:, s, n * 512 : (n + 1) * 512],
                            xt[:, s, n * 512 : (n + 1) * 512],
                            pc[:, :],
                        )
                        if with_bias:
                            pc2 = pcpool.tile([128, 512], FP32, tag="pc2")
                            nc.tensor.matmul(
                                pc2[:, :],
                                lhsT=gT[:, :],
                                rhs=eb_sb[:, n * 512 : (n + 1) * 512],
                                start=True,
                                stop=True,
                            )
                            nc.vector.tensor_add(
                                out_sb[:, s, n * 512 : (n + 1) * 512],
                                out_sb[:, s, n * 512 : (n + 1) * 512],
                                pc2[:, :],
                            )

                nc.scalar.dma_start(
                    out=y[u * BLK * 128 : (u + 1) * BLK * 128, :].rearrange(
                        "(s p) i -> p s i", p=128
                    ),
                    in_=out_sb[:, :, :],
                )

    nc.compile()
    return nc




# ---------------------------------------------------- pipelined split (A/B)
def build_phaseA_nc(k=2):
    """W-shard reduce + gating pre-work in one launch (PE hidden under DMA).

    Outputs: wsum [32,128] (this core's ws rows, flat) and
    gt [8, 32, 128]: per token-tile s, G^T = top-k-masked softmax transposed.
    """
    nc = bacc.Bacc(
        "TRN2", target_bir_lowering=False, debug=False, num_devices=N_CORES
    )
    w = nc.dram_tensor("w", [EPC * I, O], FP32, kind="ExternalInput")
    x = nc.dram_tensor("x", [TPC, I], FP32, kind="ExternalInput")
    gwt = nc.dram_tensor("gwt", [128, I // 128, E], FP32, kind="ExternalInput")
    gb = nc.dram_tensor("gb", [1, E], FP32, kind="ExternalInput")
    wsum = nc.dram_tensor("wsum", [32, 128], FP32, kind="ExternalOutput")
    gt = nc.dram_tensor("gt", [TPC // 128, E, 128], FP32, kind="ExternalOutput")

    SUB = 4
    n_dma = (EPC * I) // (SUB * 128)
    NCH = I // 128
    BLK = 4
    n_blk = TPC // (BLK * 128)

    with tile.TileContext(nc) as tc:
        with (
            tc.tile_pool(name="wt", bufs=4) as wpool,
            tc.tile_pool(name="const", bufs=1) as cpool,
            tc.tile_pool(name="xin", bufs=2) as xpool,
            tc.tile_pool(name="xt", bufs=2) as xtpool,
            tc.tile_pool(name="lgs", bufs=2) as lgspool,
            tc.tile_pool(name="gtsb", bufs=4) as gtpool,
            tc.tile_pool(name="sm", bufs=6) as smpool,
            tc.tile_pool(name="ptr", bufs=2, space="PSUM") as ptrpool,
            tc.tile_pool(name="plg", bufs=1, space="PSUM") as plgpool,
            tc.tile_pool(name="psm", bufs=2, space="PSUM") as psmpool,
            tc.tile_pool(name="pgt2", bufs=1, space="PSUM") as pgtpool,
        ):
            ident = cpool.tile([128, 128], FP32)
            make_identity(nc, ident[:, :])
            ones = cpool.tile([1, 512], FP32)
            nc.vector.memset(ones[:, :], 1.0)
            gwt_sb = cpool.tile([128, NCH, E], FP32)
            nc.scalar.dma_start(out=gwt_sb[:, :, :], in_=gwt[:, :, :])
            gb_sb = cpool.tile([1, E], FP32)
            nc.scalar.dma_start(out=gb_sb[:, :], in_=gb[:, :])

            # ---- W reduce (sync ring, DVE/ACT row sums)
            ws_acc = cpool.tile([128, 32], FP32)
            junk = cpool.tile([128, O], FP32)
            for u in range(n_dma):
                wt = wpool.tile([128, SUB, O], FP32)
                nc.sync.dma_start(
                    out=wt[:, :, :],
                    in_=w[u * SUB * 128 : (u + 1) * SUB * 128, :].rearrange(
                        "(s p) o -> p s o", p=128
                    ),
                )
                for s in range(SUB):
                    t = u * SUB + s
                    if s == 0:
                        nc.vector.reduce_sum(
                            ws_acc[:, t : t + 1],
                            wt[:, s, :],
                            axis=mybir.AxisListType.X,
                        )
                    else:
                        nc.scalar.activation(
                            junk[:, :],
                            wt[:, s, :],
                            mybir.ActivationFunctionType.Copy,
                            bias=0.0,
                            scale=1.0,
                            accum_out=ws_acc[:, t : t + 1],
                        )

            # ---- gating pre-work (x on scalar ring, PE under the W stream)
            for u in range(n_blk):
                xt = xpool.tile([128, BLK, I], FP32)
                nc.scalar.dma_start(
                    out=xt[:, :, :],
                    in_=x[u * BLK * 128 : (u + 1) * BLK * 128, :].rearrange(
                        "(s p) i -> p s i", p=128
                    ),
                )
                xT_sb = xtpool.tile([128, NCH, BLK * 128], FP32)
                for s in range(BLK):
                    for h in range(NCH // 4):
                        ptr_t = ptrpool.tile([128, 4, 128], FP32)
                        for q in range(4):
                            c = 4 * h + q
                            nc.tensor.transpose(
                                ptr_t[:, q, :],
                                xt[:, s, c * 128 : (c + 1) * 128],
                                ident[:, :],
                            )
                        nc.vector.tensor_copy(
                            out=xT_sb[
                                :, 4 * h : 4 * h + 4, s * 128 : (s + 1) * 128
                            ],
                            in_=ptr_t[:, :, :],
                        )
                lgT = plgpool.tile([E, BLK * 128], FP32)
                for c in range(NCH):
                    nc.tensor.matmul(
                        lgT[:, :],
                        lhsT=gwt_sb[:, c, :],
                        rhs=xT_sb[:, c, :],
                        start=(c == 0),
                        stop=False,
                    )
                nc.tensor.matmul(
                    lgT[:, :],
                    lhsT=gb_sb[:1, :],
                    rhs=ones[:1, :],
                    start=False,
                    stop=True,
                )
                lgT_sb = lgspool.tile([E, BLK * 128], FP32)
                nc.any.tensor_copy(out=lgT_sb[:, :], in_=lgT[:, :])

                for s in range(BLK):
                    lg = psmpool.tile([128, E], FP32, tag="lg")
                    nc.tensor.transpose(
                        lg[:, :],
                        lgT_sb[:, s * 128 : (s + 1) * 128],
                        ident[:E, :E],
                    )
                    ex = smpool.tile([128, E], FP32, tag="ex")
                    ssum = smpool.tile([128, 1], FP32, tag="ssum")
                    nc.scalar.activation(
                        ex[:, :],
                        lg[:, :],
                        mybir.ActivationFunctionType.Exp,
                        bias=0.0,
                        scale=1.0,
                        accum_out=ssum[:, :],
                    )
                    rcp = smpool.tile([128, 1], FP32, tag="rcp")
                    nc.vector.reciprocal(rcp[:, :], ssum[:, :])
                    p = smpool.tile([128, E], FP32, tag="p")
                    nc.vector.tensor_scalar_mul(p[:, :], ex[:, :], rcp[:, :])
                    mx = smpool.tile([128, 8], FP32, tag="mx")
                    nc.vector.max(out=mx[:, :], in_=p[:, :])
                    if k < 8:
                        nc.vector.memset(mx[:, k:], 0.0)
                    pz = smpool.tile([128, E], FP32, tag="pz")
                    nc.vector.match_replace(
                        out=pz[:, :],
                        in_to_replace=mx[:, :],
                        in_values=p[:, :],
                        imm_value=0.0,
                    )
                    g = smpool.tile([128, E], FP32, tag="g")
                    nc.vector.tensor_sub(g[:, :], p[:, :], pz[:, :])
                    pgt = pgtpool.tile([E, 128], FP32)
                    nc.tensor.transpose(pgt[:, :], g[:, :], ident[:, :])
                    gT = gtpool.tile([E, 128], FP32)
                    nc.any.tensor_copy(out=gT[:, :], in_=pgt[:, :])
                    nc.scalar.dma_start(
                        out=gt[u * BLK + s, :, :], in_=gT[:, :]
                    )

            wsT = psmpool.tile([32, 128], FP32, tag="lg")
            nc.tensor.transpose(wsT[:, :], ws_acc[:, :], ident[:, :])
            wsT_sb = cpool.tile([32, 128], FP32)
            nc.any.tensor_copy(out=wsT_sb[:, :], in_=wsT[:, :])
            nc.sync.dma_start(out=wsum[:, :], in_=wsT_sb[:, :])

    nc.compile()
    return nc


def build_phaseB_nc(with_bias=False):
    """Lean combine: y = x * (gt.T @ ws) [+ gt.T @ eb]. DMA-bound."""
    nc = bacc.Bacc(
        "TRN2", target_bir_lowering=False, debug=False, num_devices=N_CORES
    )
    x = nc.dram_tensor("x", [TPC, I], FP32, kind="ExternalInput")
    ws = nc.dram_tensor("ws", [E, I], FP32, kind="ExternalInput")
    gt = nc.dram_tensor("gt", [TPC // 128, E, 128], FP32, kind="ExternalInput")
    if with_bias:
        eb = nc.dram_tensor("eb", [E, O], FP32, kind="ExternalInput")
    y = nc.dram_tensor("y", [TPC, O], FP32, kind="ExternalOutput")

    BLK = 4
    n_blk = TPC // (BLK * 128)

    with tile.TileContext(nc) as tc:
        with (
            tc.tile_pool(name="const", bufs=1) as cpool,
            tc.tile_pool(name="xin", bufs=2) as xpool,
            tc.tile_pool(name="oout", bufs=2) as opool,
            tc.tile_pool(name="pc", bufs=4, space="PSUM") as pcpool,
        ):
            ws_sb = cpool.tile([E, I], FP32)
            nc.sync.dma_start(out=ws_sb[:, :], in_=ws[:, :])
            gt_sb = cpool.tile([E, TPC // 128, 128], FP32)
            nc.sync.dma_start(
                out=gt_sb[:, :, :],
                in_=gt[:, :, :].rearrange("s e p -> e s p"),
            )
            if with_bias:
                eb_sb = cpool.tile([E, O], FP32)
                nc.sync.dma_start(out=eb_sb[:, :], in_=eb[:, :])

            # HAM warm-up: ~4us of junk transposes during the DMA ramp so
            # the combine matmuls run at 2.4 GHz instead of cold 1.2
            warm_id = cpool.tile([128, 128], FP32)
            make_identity(nc, warm_id[:, :])
            for wi in range(12):
                pwarm = pcpool.tile([128, 512], FP32, tag="pc")
                nc.tensor.transpose(
                    pwarm[:, :128], warm_id[:, :], warm_id[:, :]
                )

            for u in range(n_blk):
                xt = xpool.tile([128, BLK, I], FP32)
                nc.scalar.dma_start(
                    out=xt[:, :, :],
                    in_=x[u * BLK * 128 : (u + 1) * BLK * 128, :].rearrange(
                        "(s p) i -> p s i", p=128
                    ),
                )
                out_sb = opool.tile([128, BLK, O], FP32)
                for s in range(BLK):
                    t_idx = u * BLK + s
                    for n in range(O // 512):
                        pc = pcpool.tile([128, 512], FP32, tag="pc")
                        nc.tensor.matmul(
                            pc[:, :],
                            lhsT=gt_sb[:, t_idx, :],
                            rhs=ws_sb[:, n * 512 : (n + 1) * 512],
                            start=True,
                            stop=True,
                        )
                        nc.vector.tensor_mul(
                            out_sb[:, s, n * 512 : (n + 1) * 512],
                            xt[:, s, n * 512 : (n + 1) * 512],
                            pc[:, :],
                        )
                        if with_bias:
                            pc2 = pcpool.tile([128, 512], FP32, tag="pc2")
                            nc.tensor.matmul(
                                pc2[:, :],
                                lhsT=gt_sb[:, t_idx, :],
                                rhs=eb_sb[:, n * 512 : (n + 1) * 512],
                                start=True,
                                stop=True,
                            )
                            nc.vector.tensor_add(
                                out_sb[:, s, n * 512 : (n + 1) * 512],
                                out_sb[:, s, n * 512 : (n + 1) * 512],
                                pc2[:, :],
                            )
                nc.scalar.dma_start(
                    out=y[u * BLK * 128 : (u + 1) * BLK * 128, :].rearrange(
                        "(s p) i -> p s i", p=128
                    ),
                    in_=out_sb[:, :, :],
                )

    nc.compile()
    return nc


def _get_nc(name, builder, *args):
    key = (name,) + args
    if key not in _NC_CACHE:
        _NC_CACHE[key] = builder(*args)
    return _NC_CACHE[key]


# ------------------------------------------------------------------ host glue
def kernel(**inputs):
    x = np.ascontiguousarray(inputs["x"], dtype=np.float32)
    W = np.ascontiguousarray(inputs["experts_weights"], dtype=np.float32)
    ebias = np.ascontiguousarray(inputs["experts_bias"], dtype=np.float32)
    gw = np.ascontiguousarray(inputs["gate_weight"], dtype=np.float32)
    gbias = np.ascontiguousarray(inputs["gate_bias"], dtype=np.float32)
    k = int(inputs["topk"])
    assert x.shape == (B, I) and W.shape == (E, I, O)
    assert 1 <= k <= 8

    with_bias = bool(np.any(ebias))
    cores = list(range(N_CORES))
    LAST_RESULTS.clear()

    gwt8 = np.ascontiguousarray(
        gw.T.reshape(I // 128, 128, E).transpose(1, 0, 2)
    )

    # launch A: W-shard reduce + gating pre-work (PE hidden under W DMA)
    ncA = _get_nc("phaseA", build_phaseA_nc, k)
    inA = [
        {
            "w": W[d * EPC : (d + 1) * EPC].reshape(EPC * I, O),
            "x": x[d * TPC : (d + 1) * TPC],
            "gwt": gwt8,
            "gb": gbias.reshape(1, E),
        }
        for d in cores
    ]
    rA = run_bass_kernel_spmd(ncA, inA, core_ids=cores, trace=TRACE)
    LAST_RESULTS.append(rA)
    ws_full = np.ascontiguousarray(
        np.concatenate(
            [rA.results[d]["wsum"].reshape(EPC, I) for d in cores], axis=0
        )
    )

    # launch B: lean combine y = x * (G @ ws) [+ G @ eb]
    ncB = _get_nc("phaseB", build_phaseB_nc, with_bias)
    inB = []
    for d in cores:
        m = {
            "x": x[d * TPC : (d + 1) * TPC],
            "ws": ws_full,
            "gt": rA.results[d]["gt"],
        }
        if with_bias:
            m["eb"] = ebias
        inB.append(m)
    rB = run_bass_kernel_spmd(ncB, inB, core_ids=cores, trace=TRACE)
    LAST_RESULTS.append(rB)
    out = np.concatenate([rB.results[d]["y"] for d in cores], axis=0)
    return out
